# revision 14
# baseline (speedup 1.0000x reference)
"""DeltaEncoder (delta -> BatchNorm(eval) -> Linear(1,O) -> LIF scan over O) on 8 TRN2 cores.

Strategy (pure data parallel over batch B=32 -> 4 per core):
  * Host folds BN (eval) + Linear(1,O) + the 1/TAU charge factor into per-o
    scalars  A[o] = inv*w[o]/TAU,  C[o] = ((bn_b - mu*inv)*w[o] + b[o])/TAU,
    so the per-step membrane charge is  h = (1-1/TAU)*v + (delta*A[o] + C[o]).
  * Host pre-transposes the per-core input to [(b f), t] so the device sees
    elements as [128 partitions = (b%2, f), free = (b//2, t)] with t contiguous.
  * Device computes delta along t once, then runs the 64-step LIF scan with a
    single fused custom DVE instruction per step:
        h' = (h < 1) * (h * (1-1/TAU)) + (delta * A_o + C_o)
    (state update incl. hard reset + charge, one pass at 1 elem/lane/cycle).
  * Spikes s_o = (h_o >= 1) are extracted on a second engine into a mega tile
    covering STEPS_PER_DMA o-steps, then stored with one fully-contiguous DMA
    per group into a blocked DRAM layout [p=(b1,f), o, g, t]; the host
    permutes back to [B, O, F, T].
Output: float32 spikes [B, O, F, T].
"""

import numpy as np

# problem shapes (hardcoded per contract)
_B, _T, _F, _O = 32, 512, 64, 64
_NC = 8
_BL = _B // _NC          # 4 batches per core
_G = (_BL * _F) // 128   # 2 free-dim groups of 128 (b,f) rows
_P = 128
_TAU = 2.0
_EPS = 1e-5

_LIF_OP_NAME = "LIF_STEP_ANT_RT"

# which engine extracts spikes: "gpsimd" | "dve" | "act"
SPIKE_ENGINE = "act"
STEPS_PER_DMA = 8

_MODULE_CACHE = {}


def _register_lif_op():
    """Register the fused LIF-step custom DVE op (idempotent)."""
    import concourse.dve_ops as dve_ops
    from concourse.dve_spec import (
        C0, C1, C2, One, Spec, Src0, Src1, _has_src1, lower,
    )
    from concourse.dve_uop import DveOpSpec

    for op in dve_ops.OPS:
        if op.name == _LIF_OP_NAME:
            return op

    def _ref(in0, in1, s0, s1, imm2):
        in0 = np.asarray(in0, np.float32)
        in1 = np.asarray(in1, np.float32)
        keep = (in0 < np.float32(1.0)).astype(np.float32)
        return (
            keep * (in0 * np.float32(imm2))
            + (in1 * np.float32(s0) + np.float32(s1))
        ).astype(np.float32)

    body = (Src0 < One) * (Src0 * C2) + (Src1 * C0 + C1)
    spec = Spec(body=body, reference=_ref)

    row = dve_ops._CUSTOM_DVE_ROW_BASE + len(dve_ops.OPS)
    assert row < 0x20, "no free custom-DVE opcode rows"
    shas = {}
    for ver in ("v3", "v4"):
        uops = lower(spec, ver=ver)
        shas[ver] = DveOpSpec(
            name=_LIF_OP_NAME, opcode=row, uops=uops, rd1_en=_has_src1(spec)
        ).sha(ver)

    op = dve_ops.DveOp(_LIF_OP_NAME, spec, subdim=False, uops_sha=shas)
    dve_ops.OPS.append(op)
    dve_ops._SUB_OPCODE_FOR_NAME[op.name] = row
    dve_ops.CUSTOM_DVE_SPECS[op.name] = spec
    return op


def _build_module(spike_engine: str, reps: int = 1, variant: str = "full"):
    """Build the Bass/Tile module (one NeuronCore program, SPMD across 8).

    reps > 1 wraps the whole computation in a device-side loop — used only
    for differential wall-clock timing (output is rewritten identically).
    variant: "full" | "scanonly" (no spikes/stores) | "nostore" (no DMA out)
             | "noscan" (memset spikes + stores only) — bench-only bisection.
    """
    import concourse.bacc as bacc
    import concourse.mybir as mybir
    from concourse.tile import TileContext

    lif_op = _register_lif_op()

    nc = bacc.Bacc(
        "TRN2",
        target_bir_lowering=False,
        debug=False,
        enable_asserts=False,
        num_devices=_NC,
    )
    f32 = mybir.dt.float32

    NFREE = _G * _T          # 1024
    SPD = STEPS_PER_DMA
    # store-group sizes: SPD-steps groups, tapered at the end to shrink the
    # kernel tail (last sigmoid + small final DMA instead of one big one)
    groups = []
    rem = _O
    while rem > 2 * SPD:
        groups.append(SPD)
        rem -= SPD
    while rem > 0:
        g_sz = max(1, rem // 2) if rem > 2 else 1
        groups.append(g_sz)
        rem -= g_sz

    x_in = nc.dram_tensor("x_bft", [_BL * _F, _T], f32, kind="ExternalInput").ap()
    a_in = nc.dram_tensor("a_coef", [_P, _O], f32, kind="ExternalInput").ap()
    c_in = nc.dram_tensor("c_coef", [_P, _O], f32, kind="ExternalInput").ap()
    # blocked layout: [p=(b1,f), o, g, t]; host permutes to [b=2g+b1, o, f, t]
    out = nc.dram_tensor(
        "spikes", [_P, _O, _G, _T], f32, kind="ExternalOutput"
    ).ap()
    out2 = out.rearrange("p o g t -> p (o g t)")  # [128, O*NFREE], contiguous rows

    with TileContext(nc) as tc:
        with (
            tc.tile_pool(name="const", bufs=1) as cpool,
            tc.tile_pool(name="xd", bufs=1) as xpool,
            tc.tile_pool(name="state", bufs=4) as hpool,
            tc.tile_pool(name="spk", bufs=3) as spool,
        ):

            def body():
                a_t = cpool.tile([_P, _O], f32, tag="a")
                c_t = cpool.tile([_P, _O], f32, tag="c")
                nc.sync.dma_start(out=a_t[:], in_=a_in[:])
                nc.sync.dma_start(out=c_t[:], in_=c_in[:])

                x_t = xpool.tile([_P, NFREE], f32, tag="x")
                d_t = xpool.tile([_P, NFREE], f32, tag="d")
                x3 = x_t[:].rearrange("p (g t) -> p g t", g=_G)
                d3 = d_t[:].rearrange("p (g t) -> p g t", g=_G)
                nc.sync.dma_start(
                    out=x3, in_=x_in.rearrange("(g p) t -> p g t", p=_P)
                )

                # delta along t: d[...,0] = 0 ; d[...,1:] = x[...,1:] - x[...,:-1]
                nc.vector.memset(d3[:, :, 0:1], 0.0)
                nc.vector.tensor_sub(
                    out=d3[:, :, 1:_T], in0=x3[:, :, 1:_T], in1=x3[:, :, 0 : _T - 1]
                )

                h_prev = hpool.tile([_P, NFREE], f32, tag="h")
                nc.vector.memset(h_prev[:], 0.0)

                sigb = None
                if spike_engine == "act" and variant not in ("scanonly", "noscan"):
                    sigb = cpool.tile([_P, 1], f32, tag="sigb")
                    nc.vector.memset(sigb[:], -(2.0**100))

                decay = 1.0 - 1.0 / _TAU
                o_base = 0
                for g_sz in groups:
                    s_mega = spool.tile([_P, g_sz * NFREE], f32, tag="s")
                    for oi in range(g_sz):
                        o = o_base + oi
                        if variant != "noscan":
                            h_new = hpool.tile([_P, NFREE], f32, tag="h")
                            nc.vector._custom_dve(
                                lif_op,
                                out=h_new[:],
                                in0=h_prev[:],
                                in1=d_t[:],
                                s0=a_t[:, o : o + 1],
                                s1=c_t[:, o : o + 1],
                                imm2=decay,
                            )
                        else:
                            h_new = h_prev
                        if variant == "scanonly":
                            h_prev = h_new
                            continue
                        s_sl = s_mega[:, oi * NFREE : (oi + 1) * NFREE]
                        if variant == "noscan":
                            nc.vector.memset(s_sl, 0.0)
                        elif spike_engine == "gpsimd":
                            nc.gpsimd.tensor_single_scalar(
                                s_sl, h_new[:], 1.0, mybir.AluOpType.is_ge
                            )
                        elif spike_engine == "act":
                            # sigmoid(2^100*(h-1)) saturates to exactly 0.0/1.0
                            nc.scalar.activation(
                                s_sl,
                                h_new[:],
                                mybir.ActivationFunctionType.Sigmoid,
                                bias=sigb[:],
                                scale=2.0**100,
                            )
                        else:
                            nc.vector.tensor_single_scalar(
                                s_sl, h_new[:], 1.0, mybir.AluOpType.is_ge
                            )
                        h_prev = h_new
                    if variant not in ("scanonly", "nostore"):
                        lo = o_base * NFREE
                        hi = (o_base + g_sz) * NFREE
                        nc.sync.dma_start(out=out2[:, lo:hi], in_=s_mega[:])
                    o_base += g_sz

            if reps == 1:
                body()
            else:
                with tc.For_i(0, reps, 1):
                    body()

    nc.finalize()
    return nc


def _get_module(spike_engine: str):
    if spike_engine not in _MODULE_CACHE:
        _MODULE_CACHE[spike_engine] = _build_module(spike_engine)
    return _MODULE_CACHE[spike_engine]


def _prepare_inputs(inputs, enc_w, enc_b, bn_w, bn_b, bn_mean, bn_var):
    """Host-side marshalling: scalar folding + per-core shard/transpose."""
    x = np.ascontiguousarray(np.asarray(inputs, np.float32))
    w = np.asarray(enc_w, np.float32).reshape(_O)
    b = np.asarray(enc_b, np.float32).reshape(_O)
    bw = np.float64(np.asarray(bn_w).reshape(())[()])
    bb = np.float64(np.asarray(bn_b).reshape(())[()])
    bm = np.float64(np.asarray(bn_mean).reshape(())[()])
    bv = np.float64(np.asarray(bn_var).reshape(())[()])

    inv = bw / np.sqrt(bv + _EPS)
    beta = bb - bm * inv
    A = (inv * w.astype(np.float64) / _TAU).astype(np.float32)
    C = (((beta * w.astype(np.float64)) + b.astype(np.float64)) / _TAU).astype(
        np.float32
    )
    a_b = np.ascontiguousarray(np.broadcast_to(A, (_P, _O)))
    c_b = np.ascontiguousarray(np.broadcast_to(C, (_P, _O)))

    in_maps = []
    for core in range(_NC):
        xc = x[core * _BL : (core + 1) * _BL]          # [4, T, F]
        xt = np.ascontiguousarray(xc.transpose(0, 2, 1)).reshape(_BL * _F, _T)
        in_maps.append({"x_bft": xt, "a_coef": a_b, "c_coef": c_b})
    return in_maps


def _unpack_core(spk_blocked: np.ndarray) -> np.ndarray:
    """[p=(b1,f), o, g, t] -> [b=2g+b1, o, f, t]."""
    v = spk_blocked.reshape(2, _F, _O, _G, _T)          # [b1, f, o, g, t]
    v = v.transpose(3, 0, 2, 1, 4)                       # [g, b1, o, f, t]
    return np.ascontiguousarray(v.reshape(_BL, _O, _F, _T))


def _run(in_maps, spike_engine=None, **spmd_kwargs):
    from concourse.bass_utils import run_bass_kernel_spmd

    eng = spike_engine or SPIKE_ENGINE
    nc = _get_module(eng)
    return run_bass_kernel_spmd(nc, in_maps, core_ids=list(range(_NC)), **spmd_kwargs)


def kernel(inputs, enc_w, enc_b, bn_w, bn_b, bn_mean, bn_var):
    in_maps = _prepare_inputs(inputs, enc_w, enc_b, bn_w, bn_b, bn_mean, bn_var)
    res = _run(in_maps)
    out = np.concatenate([_unpack_core(r["spikes"]) for r in res.results], axis=0)
    return np.ascontiguousarray(out.astype(np.float32, copy=False))


# revision 21
# speedup vs baseline: 1.0654x; 1.0654x over previous
"""DeltaEncoder (delta -> BatchNorm(eval) -> Linear(1,O) -> LIF scan over O) on 8 TRN2 cores.

Strategy (pure data parallel over batch B=32 -> 4 per core):
  * Host folds BN (eval) + Linear(1,O) + the 1/TAU charge factor into per-o
    scalars  A[o] = inv*w[o]/TAU,  C[o] = ((bn_b - mu*inv)*w[o] + b[o])/TAU,
    so the per-step membrane charge is  h = (1-1/TAU)*v + (delta*A[o] + C[o]).
  * Host pre-transposes the per-core input to [(b f), t] so the device sees
    elements as [128 partitions = (b%2, f), free = (b//2, t)] with t contiguous.
  * Device computes delta along t once, then runs the 64-step LIF scan with a
    single fused custom DVE instruction per step:
        h' = (h < 1) * (h * (1-1/TAU)) + (delta * A_o + C_o)
    (state update incl. hard reset + charge, one pass at 1 elem/lane/cycle).
  * Spikes s_o = (h_o >= 1) are extracted on a second engine into a mega tile
    covering STEPS_PER_DMA o-steps, then stored with one fully-contiguous DMA
    per group into a blocked DRAM layout [p=(b1,f), o, g, t]; the host
    permutes back to [B, O, F, T].
Output: float32 spikes [B, O, F, T].
"""

import numpy as np

# problem shapes (hardcoded per contract)
_B, _T, _F, _O = 32, 512, 64, 64
_NC = 8
_BL = _B // _NC          # 4 batches per core
_G = (_BL * _F) // 128   # 2 free-dim groups of 128 (b,f) rows
_P = 128
_TAU = 2.0
_EPS = 1e-5

_LIF_OP_NAME = "LIF_STEP_ANT_RT"

# which engine extracts spikes: "gpsimd" | "dve" | "act"
SPIKE_ENGINE = "act"
STEPS_PER_DMA = 8
# bake the folded per-o scalars into instruction immediates (saves ~7us/call
# at the cost of a content-keyed NEFF compile on first use)
USE_IMM = True

_MODULE_CACHE = {}


def _register_lif_op():
    """Register the fused LIF-step custom DVE op (idempotent)."""
    import concourse.dve_ops as dve_ops
    from concourse.dve_spec import (
        C0, C1, C2, One, Spec, Src0, Src1, _has_src1, lower,
    )
    from concourse.dve_uop import DveOpSpec

    for op in dve_ops.OPS:
        if op.name == _LIF_OP_NAME:
            return op

    def _ref(in0, in1, s0, s1, imm2):
        in0 = np.asarray(in0, np.float32)
        in1 = np.asarray(in1, np.float32)
        keep = (in0 < np.float32(1.0)).astype(np.float32)
        return (
            keep * (in0 * np.float32(imm2))
            + (in1 * np.float32(s0) + np.float32(s1))
        ).astype(np.float32)

    body = (Src0 < One) * (Src0 * C2) + (Src1 * C0 + C1)
    spec = Spec(body=body, reference=_ref)

    row = dve_ops._CUSTOM_DVE_ROW_BASE + len(dve_ops.OPS)
    assert row < 0x20, "no free custom-DVE opcode rows"
    shas = {}
    for ver in ("v3", "v4"):
        uops = lower(spec, ver=ver)
        shas[ver] = DveOpSpec(
            name=_LIF_OP_NAME, opcode=row, uops=uops, rd1_en=_has_src1(spec)
        ).sha(ver)

    op = dve_ops.DveOp(_LIF_OP_NAME, spec, subdim=False, uops_sha=shas)
    dve_ops.OPS.append(op)
    dve_ops._SUB_OPCODE_FOR_NAME[op.name] = row
    dve_ops.CUSTOM_DVE_SPECS[op.name] = spec
    return op


def _build_module(
    spike_engine: str, reps: int = 1, variant: str = "full", imm_coefs=None
):
    """Build the Bass/Tile module (one NeuronCore program, SPMD across 8).

    reps > 1 wraps the whole computation in a device-side loop — used only
    for differential wall-clock timing (output is rewritten identically).
    variant: "full" | "scanonly" (no spikes/stores) | "nostore" (no DMA out)
             | "noscan" (memset spikes + stores only) — bench-only bisection.
    """
    import concourse.bacc as bacc
    import concourse.mybir as mybir
    from concourse.tile import TileContext

    lif_op = _register_lif_op()

    nc = bacc.Bacc(
        "TRN2",
        target_bir_lowering=False,
        debug=False,
        enable_asserts=False,
        num_devices=_NC,
    )
    f32 = mybir.dt.float32

    NFREE = _G * _T          # 1024
    SPD = STEPS_PER_DMA
    # store-group sizes: SPD-steps groups, tapered at the end to shrink the
    # kernel tail (last sigmoid + small final DMA instead of one big one)
    groups = []
    rem = _O
    while rem > 2 * SPD:
        groups.append(SPD)
        rem -= SPD
    while rem > 0:
        g_sz = max(1, rem // 2) if rem > 2 else 1
        groups.append(g_sz)
        rem -= g_sz

    x_in = nc.dram_tensor("x_bft", [_BL * _F, _T], f32, kind="ExternalInput").ap()
    a_in = nc.dram_tensor("a_coef", [_P, _O], f32, kind="ExternalInput").ap()
    c_in = nc.dram_tensor("c_coef", [_P, _O], f32, kind="ExternalInput").ap()
    # blocked layout: [p=(b1,f), o, g, t]; host permutes to [b=2g+b1, o, f, t]
    out = nc.dram_tensor(
        "spikes", [_P, _O, _G, _T], f32, kind="ExternalOutput"
    ).ap()
    out2 = out.rearrange("p o g t -> p (o g t)")  # [128, O*NFREE], contiguous rows

    with TileContext(nc) as tc:
        with (
            tc.tile_pool(name="const", bufs=1) as cpool,
            tc.tile_pool(name="xd", bufs=1) as xpool,
            tc.tile_pool(name="state", bufs=4) as hpool,
            tc.tile_pool(name="spk", bufs=3) as spool,
        ):

            def body():
                if variant == "empty":
                    z_t = cpool.tile([_P, 1], f32, tag="z")
                    nc.vector.memset(z_t[:], 0.0)
                    return
                a_t = c_t = None
                if imm_coefs is None:
                    a_t = cpool.tile([_P, _O], f32, tag="a")
                    c_t = cpool.tile([_P, _O], f32, tag="c")
                    nc.sync.dma_start(out=a_t[:], in_=a_in[:])
                    nc.sync.dma_start(out=c_t[:], in_=c_in[:])

                x_t = xpool.tile([_P, NFREE], f32, tag="x")
                d_t = xpool.tile([_P, NFREE], f32, tag="d")
                x3 = x_t[:].rearrange("p (g t) -> p g t", g=_G)
                d3 = d_t[:].rearrange("p (g t) -> p g t", g=_G)
                nc.sync.dma_start(
                    out=x3, in_=x_in.rearrange("(g p) t -> p g t", p=_P)
                )

                # delta along t: d[...,0] = 0 ; d[...,1:] = x[...,1:] - x[...,:-1]
                nc.vector.memset(d3[:, :, 0:1], 0.0)
                nc.vector.tensor_sub(
                    out=d3[:, :, 1:_T], in0=x3[:, :, 1:_T], in1=x3[:, :, 0 : _T - 1]
                )

                h_prev = hpool.tile([_P, NFREE], f32, tag="h")
                nc.vector.memset(h_prev[:], 0.0)

                sigb = None
                if spike_engine == "act" and variant not in ("scanonly", "noscan"):
                    sigb = cpool.tile([_P, 1], f32, tag="sigb")
                    nc.vector.memset(sigb[:], -(2.0**100))

                decay = 1.0 - 1.0 / _TAU
                o_base = 0
                for g_sz in groups:
                    s_mega = spool.tile([_P, g_sz * NFREE], f32, tag="s")
                    for oi in range(g_sz):
                        o = o_base + oi
                        if variant != "noscan":
                            h_new = hpool.tile([_P, NFREE], f32, tag="h")
                            if imm_coefs is not None:
                                s0o, s1o = float(imm_coefs[0][o]), float(imm_coefs[1][o])
                            else:
                                s0o, s1o = a_t[:, o : o + 1], c_t[:, o : o + 1]
                            nc.vector._custom_dve(
                                lif_op,
                                out=h_new[:],
                                in0=h_prev[:],
                                in1=d_t[:],
                                s0=s0o,
                                s1=s1o,
                                imm2=decay,
                            )
                        else:
                            h_new = h_prev
                        if variant == "scanonly":
                            h_prev = h_new
                            continue
                        s_sl = s_mega[:, oi * NFREE : (oi + 1) * NFREE]
                        if variant == "noscan":
                            nc.vector.memset(s_sl, 0.0)
                        elif spike_engine == "gpsimd":
                            nc.gpsimd.tensor_single_scalar(
                                s_sl, h_new[:], 1.0, mybir.AluOpType.is_ge
                            )
                        elif spike_engine == "act":
                            # sigmoid(2^100*(h-1)) saturates to exactly 0.0/1.0
                            nc.scalar.activation(
                                s_sl,
                                h_new[:],
                                mybir.ActivationFunctionType.Sigmoid,
                                bias=sigb[:],
                                scale=2.0**100,
                            )
                        else:
                            nc.vector.tensor_single_scalar(
                                s_sl, h_new[:], 1.0, mybir.AluOpType.is_ge
                            )
                        h_prev = h_new
                    if variant not in ("scanonly", "nostore"):
                        lo = o_base * NFREE
                        hi = (o_base + g_sz) * NFREE
                        nc.sync.dma_start(out=out2[:, lo:hi], in_=s_mega[:])
                    o_base += g_sz

            if reps == 1:
                body()
            else:
                with tc.For_i(0, reps, 1):
                    body()

    nc.finalize()
    return nc


def _get_module(spike_engine: str, imm_coefs=None):
    if imm_coefs is not None:
        key = (spike_engine, imm_coefs[0].tobytes(), imm_coefs[1].tobytes())
    else:
        key = spike_engine
    if key not in _MODULE_CACHE:
        _MODULE_CACHE[key] = _build_module(spike_engine, imm_coefs=imm_coefs)
    return _MODULE_CACHE[key]


def _prepare_inputs(inputs, enc_w, enc_b, bn_w, bn_b, bn_mean, bn_var):
    """Host-side marshalling: scalar folding + per-core shard/transpose."""
    x = np.ascontiguousarray(np.asarray(inputs, np.float32))
    w = np.asarray(enc_w, np.float32).reshape(_O)
    b = np.asarray(enc_b, np.float32).reshape(_O)
    bw = np.float64(np.asarray(bn_w).reshape(())[()])
    bb = np.float64(np.asarray(bn_b).reshape(())[()])
    bm = np.float64(np.asarray(bn_mean).reshape(())[()])
    bv = np.float64(np.asarray(bn_var).reshape(())[()])

    inv = bw / np.sqrt(bv + _EPS)
    beta = bb - bm * inv
    A = (inv * w.astype(np.float64) / _TAU).astype(np.float32)
    C = (((beta * w.astype(np.float64)) + b.astype(np.float64)) / _TAU).astype(
        np.float32
    )
    a_b = np.ascontiguousarray(np.broadcast_to(A, (_P, _O)))
    c_b = np.ascontiguousarray(np.broadcast_to(C, (_P, _O)))

    in_maps = []
    for core in range(_NC):
        xc = x[core * _BL : (core + 1) * _BL]          # [4, T, F]
        xt = np.ascontiguousarray(xc.transpose(0, 2, 1)).reshape(_BL * _F, _T)
        in_maps.append({"x_bft": xt, "a_coef": a_b, "c_coef": c_b})
    return in_maps


def _unpack_core(spk_blocked: np.ndarray) -> np.ndarray:
    """[p=(b1,f), o, g, t] -> [b=2g+b1, o, f, t]."""
    v = spk_blocked.reshape(2, _F, _O, _G, _T)          # [b1, f, o, g, t]
    v = v.transpose(3, 0, 2, 1, 4)                       # [g, b1, o, f, t]
    return np.ascontiguousarray(v.reshape(_BL, _O, _F, _T))


def _run(in_maps, spike_engine=None, **spmd_kwargs):
    from concourse.bass_utils import run_bass_kernel_spmd

    eng = spike_engine or SPIKE_ENGINE
    imm_coefs = None
    if USE_IMM:
        imm_coefs = (in_maps[0]["a_coef"][0], in_maps[0]["c_coef"][0])
    nc = _get_module(eng, imm_coefs)
    return run_bass_kernel_spmd(nc, in_maps, core_ids=list(range(_NC)), **spmd_kwargs)


def kernel(inputs, enc_w, enc_b, bn_w, bn_b, bn_mean, bn_var):
    in_maps = _prepare_inputs(inputs, enc_w, enc_b, bn_w, bn_b, bn_mean, bn_var)
    res = _run(in_maps)
    out = np.concatenate([_unpack_core(r["spikes"]) for r in res.results], axis=0)
    return np.ascontiguousarray(out.astype(np.float32, copy=False))


# revision 24
# speedup vs baseline: 1.1853x; 1.1125x over previous
"""DeltaEncoder (delta -> BatchNorm(eval) -> Linear(1,O) -> LIF scan over O) on 8 TRN2 cores.

Strategy (pure data parallel over batch B=32 -> 4 per core):
  * Host folds BN (eval) + Linear(1,O) + the 1/TAU charge factor into per-o
    scalars  A[o] = inv*w[o]/TAU,  C[o] = ((bn_b - mu*inv)*w[o] + b[o])/TAU,
    so the per-step membrane charge is  h = (1-1/TAU)*v + (delta*A[o] + C[o]).
  * Host pre-transposes the per-core input to [(b f), t] so the device sees
    elements as [128 partitions = (b%2, f), free = (b//2, t)] with t contiguous.
  * Device computes delta along t once, then runs the 64-step LIF scan with a
    single fused custom DVE instruction per step:
        h' = (h < 1) * (h * (1-1/TAU)) + (delta * A_o + C_o)
    (state update incl. hard reset + charge, one pass at 1 elem/lane/cycle).
  * Spikes s_o = (h_o >= 1) are extracted on a second engine into a mega tile
    covering STEPS_PER_DMA o-steps, then stored with one fully-contiguous DMA
    per group into a blocked DRAM layout [p=(b1,f), o, g, t]; the host
    permutes back to [B, O, F, T].
Output: float32 spikes [B, O, F, T].
"""

import numpy as np

# problem shapes (hardcoded per contract)
_B, _T, _F, _O = 32, 512, 64, 64
_NC = 8
_BL = _B // _NC          # 4 batches per core
_G = (_BL * _F) // 128   # 2 free-dim groups of 128 (b,f) rows
_P = 128
_TAU = 2.0
_EPS = 1e-5

_LIF_OP_NAME = "LIF_STEP_ANT_RT"

# which engine extracts spikes: "gpsimd" | "dve" | "act"
SPIKE_ENGINE = "act"
STEPS_PER_DMA = 8
# bake the folded per-o scalars into instruction immediates (saves ~7us/call
# at the cost of a content-keyed NEFF compile on first use)
USE_IMM = True

_MODULE_CACHE = {}


def _register_lif_op():
    """Register the fused LIF-step custom DVE op (idempotent)."""
    import concourse.dve_ops as dve_ops
    from concourse.dve_spec import (
        C0, C1, C2, One, Spec, Src0, Src1, _has_src1, lower,
    )
    from concourse.dve_uop import DveOpSpec

    for op in dve_ops.OPS:
        if op.name == _LIF_OP_NAME:
            return op

    def _ref(in0, in1, s0, s1, imm2):
        in0 = np.asarray(in0, np.float32)
        in1 = np.asarray(in1, np.float32)
        keep = (in0 < np.float32(1.0)).astype(np.float32)
        return (
            keep * (in0 * np.float32(imm2))
            + (in1 * np.float32(s0) + np.float32(s1))
        ).astype(np.float32)

    body = (Src0 < One) * (Src0 * C2) + (Src1 * C0 + C1)
    spec = Spec(body=body, reference=_ref)

    row = dve_ops._CUSTOM_DVE_ROW_BASE + len(dve_ops.OPS)
    assert row < 0x20, "no free custom-DVE opcode rows"
    shas = {}
    for ver in ("v3", "v4"):
        uops = lower(spec, ver=ver)
        shas[ver] = DveOpSpec(
            name=_LIF_OP_NAME, opcode=row, uops=uops, rd1_en=_has_src1(spec)
        ).sha(ver)

    op = dve_ops.DveOp(_LIF_OP_NAME, spec, subdim=False, uops_sha=shas)
    dve_ops.OPS.append(op)
    dve_ops._SUB_OPCODE_FOR_NAME[op.name] = row
    dve_ops.CUSTOM_DVE_SPECS[op.name] = spec
    return op


def _build_module(
    spike_engine: str, reps: int = 1, variant: str = "full", imm_coefs=None
):
    """Build the Bass/Tile module (one NeuronCore program, SPMD across 8).

    reps > 1 wraps the whole computation in a device-side loop — used only
    for differential wall-clock timing (output is rewritten identically).
    variant: "full" | "scanonly" (no spikes/stores) | "nostore" (no DMA out)
             | "noscan" (memset spikes + stores only) — bench-only bisection.
    """
    import concourse.bacc as bacc
    import concourse.mybir as mybir
    from concourse.tile import TileContext

    lif_op = _register_lif_op()

    nc = bacc.Bacc(
        "TRN2",
        target_bir_lowering=False,
        debug=False,
        enable_asserts=False,
        num_devices=_NC,
    )
    f32 = mybir.dt.float32

    NFREE = _G * _T          # 1024
    SPD = STEPS_PER_DMA
    # store-group sizes: SPD-steps groups, tapered at the end to shrink the
    # kernel tail (last sigmoid + small final DMA instead of one big one)
    groups = []
    rem = _O
    while rem > 2 * SPD:
        groups.append(SPD)
        rem -= SPD
    while rem > 0:
        g_sz = rem if rem <= 2 else max(2, rem // 2)
        groups.append(g_sz)
        rem -= g_sz
    assert all(g % 2 == 0 for g in groups), groups  # pairing needs even groups

    x_in = nc.dram_tensor("x_bft", [_BL * _F, _T], f32, kind="ExternalInput").ap()
    a_in = nc.dram_tensor("a_coef", [_P, _O], f32, kind="ExternalInput").ap()
    c_in = nc.dram_tensor("c_coef", [_P, _O], f32, kind="ExternalInput").ap()
    # blocked layout: [p=(b1,f), o, g, t]; host permutes to [b=2g+b1, o, f, t]
    out = nc.dram_tensor(
        "spikes", [_P, _O, _G, _T], f32, kind="ExternalOutput"
    ).ap()
    out2 = out.rearrange("p o g t -> p (o g t)")  # [128, O*NFREE], contiguous rows

    with TileContext(nc) as tc:
        with (
            tc.tile_pool(name="const", bufs=1) as cpool,
            tc.tile_pool(name="xd", bufs=1) as xpool,
            tc.tile_pool(name="state", bufs=4) as hpool,
            tc.tile_pool(name="spk", bufs=3) as spool,
        ):

            def body():
                if variant == "empty":
                    z_t = cpool.tile([_P, 1], f32, tag="z")
                    nc.vector.memset(z_t[:], 0.0)
                    return
                a_t = c_t = None
                if imm_coefs is None:
                    a_t = cpool.tile([_P, _O], f32, tag="a")
                    c_t = cpool.tile([_P, _O], f32, tag="c")
                    nc.sync.dma_start(out=a_t[:], in_=a_in[:])
                    nc.sync.dma_start(out=c_t[:], in_=c_in[:])

                x_t = xpool.tile([_P, NFREE], f32, tag="x")
                d_t = xpool.tile([_P, NFREE], f32, tag="d")
                x3 = x_t[:].rearrange("p (g t) -> p g t", g=_G)
                d3 = d_t[:].rearrange("p (g t) -> p g t", g=_G)
                nc.sync.dma_start(
                    out=x3, in_=x_in.rearrange("(g p) t -> p g t", p=_P)
                )

                # delta along t: d[...,0] = 0 ; d[...,1:] = x[...,1:] - x[...,:-1]
                # (t=0 column memset on gpsimd to keep the DVE prologue short)
                nc.gpsimd.memset(d3[:, :, 0:1], 0.0)
                nc.vector.tensor_sub(
                    out=d3[:, :, 1:_T], in0=x3[:, :, 1:_T], in1=x3[:, :, 0 : _T - 1]
                )

                sigb = None
                if spike_engine == "act" and variant not in ("scanonly", "noscan"):
                    sigb = cpool.tile([_P, 1], f32, tag="sigb")
                    nc.gpsimd.memset(sigb[:], -(2.0**100))
                h_prev = None
                if variant == "noscan":
                    h_prev = hpool.tile([_P, 2 * NFREE], f32, tag="h")
                    nc.vector.memset(h_prev[:], 0.0)

                decay = 1.0 - 1.0 / _TAU
                o_base = 0
                h_pair = None   # current [P, 2*NFREE] paired-state tile
                h_half = None   # AP of the previous step's h half
                for g_sz in groups:
                    s_mega = spool.tile([_P, g_sz * NFREE], f32, tag="s")
                    for oi in range(g_sz):
                        o = o_base + oi
                        if variant != "noscan":
                            if o % 2 == 0:
                                h_pair = hpool.tile([_P, 2 * NFREE], f32, tag="h")
                            out_ap = h_pair[:, (o % 2) * NFREE : (o % 2 + 1) * NFREE]
                            if imm_coefs is not None:
                                s0o, s1o = float(imm_coefs[0][o]), float(imm_coefs[1][o])
                            else:
                                s0o, s1o = a_t[:, o : o + 1], c_t[:, o : o + 1]
                            if o == 0:
                                # v=0: h_0 = d*A_0 + C_0 (2x-mode tensor_scalar,
                                # replaces state memset + first custom op)
                                nc.vector.tensor_scalar(
                                    out_ap,
                                    d_t[:],
                                    s0o,
                                    s1o,
                                    mybir.AluOpType.mult,
                                    mybir.AluOpType.add,
                                )
                            else:
                                nc.vector._custom_dve(
                                    lif_op,
                                    out=out_ap,
                                    in0=h_half,
                                    in1=d_t[:],
                                    s0=s0o,
                                    s1=s1o,
                                    imm2=decay,
                                )
                            h_half = out_ap
                        else:
                            h_pair = h_prev
                        if variant == "scanonly":
                            continue
                        if variant == "noscan":
                            nc.vector.memset(
                                s_mega[:, oi * NFREE : (oi + 1) * NFREE], 0.0
                            )
                            continue
                        if o % 2 == 1:
                            # one spike-extract per pair over [P, 2*NFREE]
                            s_sl2 = s_mega[:, (oi - 1) * NFREE : (oi + 1) * NFREE]
                            if spike_engine == "gpsimd":
                                nc.gpsimd.tensor_single_scalar(
                                    s_sl2, h_pair[:], 1.0, mybir.AluOpType.is_ge
                                )
                            elif spike_engine == "act":
                                # sigmoid(2^100*(h-1)) saturates to exactly 0/1
                                nc.scalar.activation(
                                    s_sl2,
                                    h_pair[:],
                                    mybir.ActivationFunctionType.Sigmoid,
                                    bias=sigb[:],
                                    scale=2.0**100,
                                )
                            else:
                                nc.vector.tensor_single_scalar(
                                    s_sl2, h_pair[:], 1.0, mybir.AluOpType.is_ge
                                )
                    if variant not in ("scanonly", "nostore"):
                        lo = o_base * NFREE
                        hi = (o_base + g_sz) * NFREE
                        nc.sync.dma_start(out=out2[:, lo:hi], in_=s_mega[:])
                    o_base += g_sz

            if reps == 1:
                body()
            else:
                with tc.For_i(0, reps, 1):
                    body()

    nc.finalize()
    return nc


def _get_module(spike_engine: str, imm_coefs=None):
    if imm_coefs is not None:
        key = (spike_engine, imm_coefs[0].tobytes(), imm_coefs[1].tobytes())
    else:
        key = spike_engine
    if key not in _MODULE_CACHE:
        _MODULE_CACHE[key] = _build_module(spike_engine, imm_coefs=imm_coefs)
    return _MODULE_CACHE[key]


def _prepare_inputs(inputs, enc_w, enc_b, bn_w, bn_b, bn_mean, bn_var):
    """Host-side marshalling: scalar folding + per-core shard/transpose."""
    x = np.ascontiguousarray(np.asarray(inputs, np.float32))
    w = np.asarray(enc_w, np.float32).reshape(_O)
    b = np.asarray(enc_b, np.float32).reshape(_O)
    bw = np.float64(np.asarray(bn_w).reshape(())[()])
    bb = np.float64(np.asarray(bn_b).reshape(())[()])
    bm = np.float64(np.asarray(bn_mean).reshape(())[()])
    bv = np.float64(np.asarray(bn_var).reshape(())[()])

    inv = bw / np.sqrt(bv + _EPS)
    beta = bb - bm * inv
    A = (inv * w.astype(np.float64) / _TAU).astype(np.float32)
    C = (((beta * w.astype(np.float64)) + b.astype(np.float64)) / _TAU).astype(
        np.float32
    )
    a_b = np.ascontiguousarray(np.broadcast_to(A, (_P, _O)))
    c_b = np.ascontiguousarray(np.broadcast_to(C, (_P, _O)))

    in_maps = []
    for core in range(_NC):
        xc = x[core * _BL : (core + 1) * _BL]          # [4, T, F]
        xt = np.ascontiguousarray(xc.transpose(0, 2, 1)).reshape(_BL * _F, _T)
        in_maps.append({"x_bft": xt, "a_coef": a_b, "c_coef": c_b})
    return in_maps


def _unpack_core(spk_blocked: np.ndarray) -> np.ndarray:
    """[p=(b1,f), o, g, t] -> [b=2g+b1, o, f, t]."""
    v = spk_blocked.reshape(2, _F, _O, _G, _T)          # [b1, f, o, g, t]
    v = v.transpose(3, 0, 2, 1, 4)                       # [g, b1, o, f, t]
    return np.ascontiguousarray(v.reshape(_BL, _O, _F, _T))


def _run(in_maps, spike_engine=None, **spmd_kwargs):
    from concourse.bass_utils import run_bass_kernel_spmd

    eng = spike_engine or SPIKE_ENGINE
    imm_coefs = None
    if USE_IMM:
        imm_coefs = (in_maps[0]["a_coef"][0], in_maps[0]["c_coef"][0])
    nc = _get_module(eng, imm_coefs)
    return run_bass_kernel_spmd(nc, in_maps, core_ids=list(range(_NC)), **spmd_kwargs)


def kernel(inputs, enc_w, enc_b, bn_w, bn_b, bn_mean, bn_var):
    in_maps = _prepare_inputs(inputs, enc_w, enc_b, bn_w, bn_b, bn_mean, bn_var)
    res = _run(in_maps)
    out = np.concatenate([_unpack_core(r["spikes"]) for r in res.results], axis=0)
    return np.ascontiguousarray(out.astype(np.float32, copy=False))
